# revision 13
# baseline (speedup 1.0000x reference)
"""Grouped MLP (MoE expert MLP, ragged token groups) on 8 TRN2 NeuronCores.

Strategy: tensor-parallel over the intermediate dim F. Every core holds a
1/8 column-slice of every expert's w1 (and the matching row-slice of w2)
resident in SBUF for the whole kernel — 128 KiB/partition for all 8
experts — and processes ALL T tokens, computing a partial fc2 output
that the host sums across the 8 cores. This gives:
  * zero weight reloads / zero expert-segment stalls on device,
  * perfectly balanced cores (identical token stream on every core),
  * exact-size token chunks (no padding waste: chunk = min(512, rest)).

Per chunk of m tokens for expert e (all in the transposed [feat, tok]
layout so weights are the PE-stationary operand):
  ps1[f, :m]  = sum_h w1sb[e][h-part, f-cols].T @ xt[h-part, m]   (4 f-tiles)
  act         = gelu(ps1)                  (Activation engine, bf16 out)
  ps2[h, :m]  = sum_f w2sb[e][f-part, h-cols].T @ act[f-part, m]  (8 h-tiles)
  yt          = bf16(ps2)                  (DVE cast)
Partial y leaves as bf16; the host upconverts (exact: bf16 is truncated
f32) and accumulates in f32.

DMA layout: x and y live in DRAM chunk-major and flat per partition
([128, sum(HT*m)]) so every chunk transfer is one contiguous 8KB-per-
partition descriptor — ~5x the per-engine DMA rate of strided 1KB
pieces. Weight loads are interleaved across the two spare DGE queues
(gpsimd SWDGE / Activation HWDGE) in first-use order so early experts'
weights land before the PE needs them; the first expert's tiles are
split in half for a faster pipeline start. The schedule ends with the
globally smallest chunk to minimize the post-matmul drain tail.

Host side: pack x.T/weights as bf16, sum the 8 partial outputs.
Compiled program cached per chunk schedule.
"""

import numpy as np
import ml_dtypes

import concourse.bass as bass
import concourse.mybir as mybir
import concourse.tile as tile
from concourse import bacc
from concourse.bass_utils import run_bass_kernel_spmd

# Problem shape (fixed by the task).
T, H, F, E = 16384, 1024, 4096, 8
NCORES = 8
FS = F // NCORES      # per-core F slice = 512
HT = H // 128         # 8 h-tiles
FT = FS // 128        # 4 f-tiles per core
CHUNK = 512           # max tokens per chunk = matmul moving-dim N

_BF16 = mybir.dt.bfloat16
_F32 = mybir.dt.float32

_cache = {}

GELU_FUNC = mybir.ActivationFunctionType.Gelu


def _schedule(counts):
    """counts[E] -> list of (expert, token_start, m) with exact sizes.

    Expert processing order puts the expert owning the globally smallest
    chunk last (and that chunk last within it) so the kernel's drain tail
    is as short as possible.
    """
    starts = np.concatenate([[0], np.cumsum([int(c) for c in counts])])
    per_e = []
    for e in range(E):
        c = int(counts[e])
        ch = []
        o = 0
        while o < c:
            m = min(CHUNK, c - o)
            ch.append((e, int(starts[e]) + o, m))
            o += m
        if ch:
            per_e.append(ch)
    if not per_e:
        return []
    # Biggest expert first: its chunks cover the FIFO drain of the single
    # weight-DMA queue, so later experts' weights always arrive in time.
    per_e.sort(key=lambda ch: -len(ch))
    # Expert whose last (smallest) chunk is globally smallest goes last so
    # the drain tail after the final matmul is minimal.
    if len(per_e) > 1:
        tail_i = min(range(1, len(per_e)), key=lambda i: per_e[i][-1][2])
        per_e.append(per_e.pop(tail_i))
    chunks = []
    for ch in per_e:
        chunks.extend(ch)
    return chunks


def _build(chunks):
    """Build + compile the SPMD program for a static chunk schedule."""
    key = (tuple(chunks), GELU_FUNC)
    if key in _cache:
        return _cache[key]

    xtot = sum(HT * m for _, _, m in chunks)
    nc = bacc.Bacc("TRN2", target_bir_lowering=False, debug=False,
                   num_devices=NCORES)
    xt_d = nc.declare_dram_parameter("xt", [128, xtot], _BF16,
                                     isOutput=False)
    w1_d = nc.declare_dram_parameter("w1s", [E, 128, HT, FS], _BF16,
                                     isOutput=False)
    w2_d = nc.declare_dram_parameter("w2s", [E, 128, FT, H], _BF16,
                                     isOutput=False)
    yt_d = nc.declare_dram_parameter("yt", [128, xtot], _BF16,
                                     isOutput=True)

    experts_used = []
    for e, _, _ in chunks:
        if e not in experts_used:
            experts_used.append(e)

    with tile.TileContext(nc) as tc:
        with (
            tc.tile_pool(name="w1", bufs=1) as w1pool,
            tc.tile_pool(name="w2", bufs=1) as w2pool,
            tc.tile_pool(name="x", bufs=3) as xpool,
            tc.tile_pool(name="act", bufs=2) as apool,
            tc.tile_pool(name="y", bufs=3) as ypool,
            tc.tile_pool(name="ps1", bufs=3, space="PSUM") as ps1pool,
            tc.tile_pool(name="ps2", bufs=3, space="PSUM") as ps2pool,
        ):
            # All experts' weight slices stay SBUF-resident, loaded on the
            # otherwise-idle gpsimd SWDGE queue ONLY (DMA queues drain
            # FIFO; mixing streams in a queue delays early items). First-
            # use order + biggest-expert-first schedule means each
            # expert's weights land well before the PE reaches them.
            w1sb = {}
            w2sb = {}
            for k, e in enumerate(experts_used):
                t1 = w1pool.tile([128, HT, FS], _BF16, tag=f"w1_{e}",
                                 name=f"w1sb{e}")
                if k == 0:
                    # split so fc1 f-tile 0 can start before the rest lands
                    half = FS // 2
                    nc.gpsimd.dma_start(t1[:, :, :half], w1_d[e][:, :, :half])
                    nc.gpsimd.dma_start(t1[:, :, half:], w1_d[e][:, :, half:])
                else:
                    nc.gpsimd.dma_start(t1[:], w1_d[e])
                t2 = w2pool.tile([128, FT, H], _BF16, tag=f"w2_{e}",
                                 name=f"w2sb{e}")
                nc.gpsimd.dma_start(t2[:], w2_d[e])
                w1sb[e] = t1
                w2sb[e] = t2

            base = 0
            first_chunk = True
            for e, off, m in chunks:
                xt = xpool.tile([128, HT * CHUNK], _BF16, tag="xt")
                if first_chunk:
                    # split so the first fc1 accumulation (h 0..3) can
                    # start while the second half is still in flight
                    hh = (HT // 2) * m
                    nc.sync.dma_start(xt[:, :hh], xt_d[:, base:base + hh])
                    nc.sync.dma_start(xt[:, hh:HT * m],
                                      xt_d[:, base + hh:base + HT * m])
                    first_chunk = False
                else:
                    nc.sync.dma_start(xt[:, :HT * m],
                                      xt_d[:, base:base + HT * m])
                act = apool.tile([128, FT, CHUNK], _BF16, tag="act")
                for f in range(FT):
                    ps = ps1pool.tile([128, CHUNK], _F32, tag="ps1")
                    for h in range(HT):
                        nc.tensor.matmul(
                            ps[:, :m],
                            w1sb[e][:, h, f * 128:(f + 1) * 128],
                            xt[:, h * m:(h + 1) * m],
                            start=(h == 0), stop=(h == HT - 1))
                    nc.scalar.activation(act[:, f, :m], ps[:, :m], GELU_FUNC)
                yt = ypool.tile([128, HT * CHUNK], _BF16, tag="yt")
                for h in range(HT):
                    ps2 = ps2pool.tile([128, CHUNK], _F32, tag="ps2")
                    for f in range(FT):
                        nc.tensor.matmul(
                            ps2[:, :m],
                            w2sb[e][:, f, h * 128:(h + 1) * 128],
                            act[:, f, :m],
                            start=(f == 0), stop=(f == FT - 1))
                    nc.vector.tensor_copy(yt[:, h * m:(h + 1) * m],
                                          ps2[:, :m])
                nc.scalar.dma_start(yt_d[:, base:base + HT * m],
                                    yt[:, :HT * m])
                base += HT * m
    nc.compile()
    _cache[key] = nc
    return nc


def _make_inputs(x, w1, w2, chunks):
    """Per-core input maps. xt is shared (x.T packed chunk-major, flat);
    weights are per-core F-slices."""
    xtT = np.ascontiguousarray(
        x.astype(ml_dtypes.bfloat16).T.reshape(HT, 128, T).transpose(1, 0, 2))
    xt = np.concatenate(
        [xtT[:, :, off:off + m].reshape(128, HT * m) for _, off, m in chunks],
        axis=1)
    w1b = w1.astype(ml_dtypes.bfloat16)
    w2b = w2.astype(ml_dtypes.bfloat16)
    in_maps = []
    for c in range(NCORES):
        w1s = np.ascontiguousarray(
            w1b[:, :, c * FS:(c + 1) * FS]
            .reshape(E, HT, 128, FS).transpose(0, 2, 1, 3))
        w2s = np.ascontiguousarray(
            w2b[:, c * FS:(c + 1) * FS, :]
            .reshape(E, FT, 128, H).transpose(0, 2, 1, 3))
        in_maps.append({"xt": xt, "w1s": w1s, "w2s": w2s})
    return in_maps


def _gather(results, chunks):
    """Sum 8 bf16 partial outputs in f32 and restore [T, H] layout."""
    acc = np.zeros((128, HT, T), np.float32)
    for c in range(NCORES):
        yb = results[c]["yt"]
        # bf16 -> f32 exactly via bit shift (bf16 is truncated f32)
        yf = (yb.view(np.uint16).astype(np.uint32) << 16).view(np.float32)
        base = 0
        for _, off, m in chunks:
            acc[:, :, off:off + m] += yf[:, base:base + HT * m].reshape(
                128, HT, m)
            base += HT * m
    return np.ascontiguousarray(acc.transpose(1, 0, 2).reshape(H, T).T)


def kernel(permuted_local_hidden_states, weight1, weight2, tokens_per_expert):
    x = np.asarray(permuted_local_hidden_states, np.float32)
    w1 = np.asarray(weight1, np.float32)
    w2 = np.asarray(weight2, np.float32)
    counts = np.asarray(tokens_per_expert).astype(np.int64)

    chunks = _schedule(counts)
    nc = _build(chunks)
    in_maps = _make_inputs(x, w1, w2, chunks)
    res = run_bass_kernel_spmd(nc, in_maps, list(range(NCORES)))
    return _gather(res.results, chunks)
